# revision 59
# baseline (speedup 1.0000x reference)
"""Distributed Trainium2 Bass kernel for nn_Attention_87368224735328.

reference:
    score = einsum("bqd,bkd->bqk", enc_outputs, atten_outputs)   # [B,S1,S2]
    alignment = softmax(score, axis=-1)                          # over S2
    out = einsum("bqk,bqd->bkd", alignment, enc_outputs + enc_residual)

Sharding: 8 cores = (batch b in 0..3) x (S2-half in 0..1). Each core computes
its local [S1, S2/2] score block, local softmax row-stats (max / sum-exp over
its S2 half), exchanges the tiny [S1] stats with its partner core, and runs
the second GEMM fully locally (contraction over S1 is complete on every
core). Output shard: [S2/2, D] -> out[b, half].

Precision strategy (rel-err gate is 2e-2; measured ~8e-3):
  * Both GEMMs run on the TensorEngine in fp8-e4m3 DoubleRow mode
    (0.5 cycles/row vs fp16's 1.0, and 2 contraction k-tiles packed per
    instruction), with error compensation:
  * GEMM1 (scores): host splits Q^T and K^T into exact fp8 hi + lo
    residual parts (RNE). Three pair-packed DoubleRow passes accumulate
    qh*kh + ql*kh + qh*kl into one PSUM group -> score error ~1e-1 abs,
    which near-one-hot softmax tolerates (sensitivity a(1-a)).
  * softmax numerator E8 = fp8(exp(S - m_loc)) via ACT; the row-sum Z8 is
    taken POST-quantization (ACT self-copy accum) so the normalization
    a = E8/Z8 cancels the quantization error of the dominant entries.
  * GEMM2: E8 (x) fp8(V'c) hi + E8 (x) fp8 lo-residual, both pair-packed
    into one PSUM group. V'c = (enc+res)*c, c = exp(m_loc-m_glob)/Z_glob.

Stats exchange: one-chip 8-core AllGather with the partner's slice
extracted rank-agnostically via a host-provided one-hot mask; split in
three (SPLITS) so each AllGather hides under TensorEngine work.
"""

import os
import numpy as np
import ml_dtypes

K_ER = int(os.environ.get("K_ER", "1"))      # 0: er8-15 in bulk; 1: at exchange splits
K_Z8 = int(os.environ.get("K_Z8", "0"))      # 0: z8 all ACT; 1: alternate ACT/DVE
K_XM = int(os.environ.get("K_XM", "14"))     # ex1 math emission tile (qi+1)

from concourse import bacc, mybir, tile
from concourse.bass_utils import run_bass_kernel_spmd

B, S, D = 4, 2048, 1024
S2L = S // 2          # local S2 columns per core
NQT = S // 128        # 16 q tiles (S1)
NT = D // 256         # 4 d-chunk pairs for GEMM1 DoubleRow
NQP = NQT // 2        # 8 q-tile pairs for GEMM2 DoubleRow
SPLITS = (8, 12)      # stats-exchange boundaries (in q tiles)
E4NP = ml_dtypes.float8_e4m3
FP8 = mybir.dt.float8e4
FP16 = mybir.dt.float16
F32 = mybir.dt.float32
DR = mybir.MatmulPerfMode.DoubleRow
N_CORES = 8
RG8 = [[0, 1, 2, 3, 4, 5, 6, 7]]


def _exchange_dmas(nc, P, DRAM, negm, zloc, lo, hi, tag, use_collective):
    """AllGather all cores' (-m, z) for q tiles [lo, hi) into gath."""
    n = hi - lo
    stats_in = DRAM.tile([128, 2 * n], F32, name=f"stats_in{tag}")
    stats_out = DRAM.tile([N_CORES, 128, 2 * n], F32, name=f"stats_out{tag}")
    # sync (SP) queue: these block on z8 and must not sit ahead of compute
    # ops on ACT/Pool; by the time they fire the sync bulk stream is done
    nc.sync.dma_start(out=stats_in[:, 0:n], in_=negm[:, lo:hi])
    nc.sync.dma_start(out=stats_in[:, n:2 * n], in_=zloc[:, lo:hi])
    if use_collective:
        nc.gpsimd.collective_compute(
            "AllGather", mybir.AluOpType.bypass,
            replica_groups=RG8,
            ins=[stats_in[:, :].opt()],
            outs=[stats_out[:, :, :].opt()],
        )
    else:  # debug/sim variant: pretend every rank has our stats
        for r in range(N_CORES):
            nc.sync.dma_start(out=stats_out[r], in_=stats_in[:, :])
    gath = P.tile([128, N_CORES, 2 * n], F32, tag=f"gath{tag}",
                  name=f"gath{tag}")
    nc.sync.dma_start(out=gath[:, :, :],
                        in_=stats_out[:, :, :].rearrange("r p c -> p r c"))
    return gath


def _exchange_math(nc, P, sel_sb, negm, zloc, cs, gath, lo, hi, tag):
    """Pick the partner slice with the one-hot mask and write cs[:, lo:hi].
    Emitted LATER than the DMAs so the gather-wait never head-of-line
    blocks per-tile work in the DVE FIFO."""
    n = hi - lo
    # partner slice = sum_r sel[r] * gath[r]  (sel is one-hot at partner)
    acc = P.tile([128, 2 * n], F32, tag=f"acc{tag}", name=f"acc{tag}")
    nc.vector.tensor_scalar_mul(out=acc[:, :], in0=gath[:, 0, :],
                                scalar1=sel_sb[:, 0:1])
    for r in range(1, N_CORES):
        nc.vector.scalar_tensor_tensor(
            out=acc[:, :], in0=gath[:, r, :], scalar=sel_sb[:, r:r + 1],
            in1=acc[:, :], op0=mybir.AluOpType.mult, op1=mybir.AluOpType.add)

    # all in negated-max terms: ng = -m_glob = min(negm0, negm1);
    # t_i = exp(ng - negm_i) = exp(m_i - m_glob)
    n0, z0 = negm[:, lo:hi], zloc[:, lo:hi]
    n1, z1 = acc[:, 0:n], acc[:, n:2 * n]
    ng = P.tile([128, n], F32, tag=f"ng{tag}", name=f"ng{tag}")
    t0 = P.tile([128, n], F32, tag=f"t0{tag}", name=f"t0{tag}")
    t1 = P.tile([128, n], F32, tag=f"t1{tag}", name=f"t1{tag}")
    zg = P.tile([128, n], F32, tag=f"zg{tag}", name=f"zg{tag}")
    rz = P.tile([128, n], F32, tag=f"rz{tag}", name=f"rz{tag}")
    nc.vector.tensor_tensor(out=ng[:, :], in0=n0, in1=n1,
                            op=mybir.AluOpType.min)
    nc.vector.tensor_sub(out=t0[:, :], in0=ng[:, :], in1=n0)
    nc.vector.tensor_sub(out=t1[:, :], in0=ng[:, :], in1=n1)
    nc.scalar.activation(out=t0[:, :], in_=t0[:, :],
                         func=mybir.ActivationFunctionType.Exp)
    nc.scalar.activation(out=t1[:, :], in_=t1[:, :],
                         func=mybir.ActivationFunctionType.Exp)
    nc.vector.tensor_mul(out=zg[:, :], in0=t0[:, :], in1=z0)
    nc.vector.tensor_mul(out=t1[:, :], in0=t1[:, :], in1=z1)
    nc.vector.tensor_add(out=zg[:, :], in0=zg[:, :], in1=t1[:, :])
    # c = exp(m_loc - m_glob) / Z_glob = t0 / Z_glob
    nc.vector.reciprocal(out=rz[:, :], in_=zg[:, :])
    nc.vector.tensor_mul(out=cs[:, lo:hi], in0=t0[:, :], in1=rz[:, :])


def _emit_vprod(nc, v16, vhi, vlo, cs, qj):
    """vhi = fp8(v16*c) on ACT; vlo = fp8(v16*c - vhi), alternating
    DVE / (DVE-scale + Pool-sub) so neither engine eats the whole burst."""
    nc.scalar.activation(
        out=vhi[:, qj, :], in_=v16[:, qj, :],
        func=mybir.ActivationFunctionType.Copy,
        scale=cs[:, qj:qj + 1])
    if qj % 2 == 0 or qj >= 12:
        nc.vector.scalar_tensor_tensor(
            out=vlo[:, qj, :], in0=v16[:, qj, :], scalar=cs[:, qj:qj + 1],
            in1=vhi[:, qj, :], op0=mybir.AluOpType.mult,
            op1=mybir.AluOpType.subtract)
    else:
        nc.vector.tensor_scalar_mul(
            out=v16[:, qj, :], in0=v16[:, qj, :], scalar1=cs[:, qj:qj + 1])
        nc.gpsimd.tensor_tensor(
            out=vlo[:, qj, :], in0=v16[:, qj, :], in1=vhi[:, qj, :],
            op=mybir.AluOpType.subtract)


def _emit_body(nc, tc, pools, qh, ql, kh, kl, er, sel, out,
               use_collective):
    P, ST, PS, OST, DRAM = pools

    # ---- persistent SBUF tensors -------------------------------
    qh_sb = P.tile([128, NT, 2, S], FP8, tag="qh", name="qh")
    ql_sb = P.tile([128, NT, 2, S], FP8, tag="ql", name="ql")
    kh_sb = P.tile([128, NT, 2, S2L], FP8, tag="kh", name="kh")
    kl_sb = P.tile([128, NT, 2, S2L], FP8, tag="kl", name="kl")
    e8 = P.tile([128, NQT, S2L], FP8, tag="e8", name="e8")
    v16 = P.tile([128, NQT, D], FP16, tag="v16", name="v16")
    vhi = P.tile([128, NQT, D], FP8, tag="vhi", name="vhi")
    vlo = P.tile([128, NQT, D], FP8, tag="vlo", name="vlo")
    negm = P.tile([128, NQT], F32, tag="negm", name="negm")
    zloc = P.tile([128, NQT], F32, tag="zloc", name="zloc")
    cs = P.tile([128, NQT], F32, tag="cs", name="cs")
    sel_sb = P.tile([128, N_CORES], F32, tag="sel", name="sel_sb")

    # ---- load GEMM1 operands ------------------------------------
    # The cost model charges HWDGE a flat ~630ns per DMA and serializes
    # all HWDGE queues, so DMA count is the currency. q operands stream
    # in three tile-aligned waves matched to GEMM1's consumption order;
    # k operands land first (every tile needs all of them).
    #   sync:   qh/ql waves, then er (enc||res) stream
    #   scalar: kh (ramp), kl (pass-2), then exchange DMAs + stores
    WAVES = ((512, 1280), (1280, 2048))
    er_tiles = []

    def _er_batch(lo_t, hi_t):
        for qi in range(lo_t, hi_t):
            er_t = ST.tile([128, 2 * D], FP16, tag="er", name=f"er{qi}")
            nc.sync.dma_start(out=er_t[:, :],
                              in_=er[qi * 128:(qi + 1) * 128, :])
            er_tiles.append(er_t)

    # Single consumption-ordered stream on the sync queue: the HWDGE
    # round-robins across queues, so anything on the scalar queue would
    # steal slots from this critical stream. The scalar queue only gets
    # the exchange DMAs (blocked on z8 until mid-GEMM1 anyway).
    nc.sync.dma_start(out=kh_sb[:, 0, :, 0:256], in_=kh[:, 0, :, 0:256])
    nc.sync.dma_start(out=qh_sb[:, 0, :, 0:256], in_=qh[:, 0, :, 0:256])
    nc.sync.dma_start(out=kh_sb[:, 0, :, 256:512], in_=kh[:, 0, :, 256:512])
    nc.sync.dma_start(out=qh_sb[:, 0, :, 256:512], in_=qh[:, 0, :, 256:512])
    nc.sync.dma_start(out=kh_sb[:, 0, :, 512:1024], in_=kh[:, 0, :, 512:1024])
    for t in range(1, NT):
        nc.scalar.dma_start(out=kh_sb[:, t, :, :], in_=kh[:, t, :, :])
        nc.sync.dma_start(out=qh_sb[:, t, :, 0:512], in_=qh[:, t, :, 0:512])
    for t in range(NT):
        nc.scalar.dma_start(out=kl_sb[:, t, :, :], in_=kl[:, t, :, :])
        nc.sync.dma_start(out=ql_sb[:, t, :, 0:512], in_=ql[:, t, :, 0:512])
    _er_batch(0, 2)
    for t in range(NT):
        nc.sync.dma_start(out=qh_sb[:, t, :, 512:1280],
                          in_=qh[:, t, :, 512:1280])
    _er_batch(2, 5)
    for t in range(NT):
        nc.sync.dma_start(out=ql_sb[:, t, :, 512:1280],
                          in_=ql[:, t, :, 512:1280])
    _er_batch(5, 8)
    for t in range(NT):
        nc.sync.dma_start(out=qh_sb[:, t, :, 1280:2048],
                          in_=qh[:, t, :, 1280:2048])
    for t in range(NT):
        nc.sync.dma_start(out=ql_sb[:, t, :, 1280:2048],
                          in_=ql[:, t, :, 1280:2048])
    if not K_ER:
        _er_batch(8, NQT)
    nc.sync.dma_start(out=sel_sb[:, :], in_=sel)

    # ---- GEMM1 + local softmax stats per q tile ----------------
    RAMP = 4
    # staircase over passes 1-2: tile qi consumes chunk s-qi at step s, so
    # each arriving chunk feeds 8 matmuls before the next one is needed
    # (chunk order matches the scalar-queue DMA order kh[0..3], kl[0..3])
    ramp_ps = [PS.tile([128, S2L], F32, tag="ps", name=f"s{qi}")
               for qi in range(RAMP)]
    for s in range(2 * NT + RAMP - 1):
        for qi in range(RAMP):
            c = s - qi
            if not 0 <= c < 2 * NT:
                continue
            t = c % NT
            ks = kh_sb if c < NT else kl_sb
            if c == 0 and qi == 0:
                # split the very first matmuls so they start on the small
                # head DMA pieces
                for kc in range(4):
                    # one start per 512-col psum zero region (kc 0 and 2)
                    nc.tensor.matmul(
                        ramp_ps[qi][:, kc * 256:(kc + 1) * 256],
                        lhsT=qh_sb[:, t, :, qi * 128:(qi + 1) * 128],
                        rhs=ks[:, t, :, kc * 256:(kc + 1) * 256],
                        start=(kc % 2 == 0), stop=False, perf_mode=DR,
                    )
                continue
            for kb in range(2):
                nc.tensor.matmul(
                    ramp_ps[qi][:, kb * 512:(kb + 1) * 512],
                    lhsT=qh_sb[:, t, :, qi * 128:(qi + 1) * 128],
                    rhs=ks[:, t, :, kb * 512:(kb + 1) * 512],
                    start=(c == 0), stop=False, perf_mode=DR,
                )
    for qi in range(NQT):
        if qi < RAMP:
            ps = ramp_ps[qi]
            passes = [(ql_sb, kh_sb)]
        else:
            ps = PS.tile([128, S2L], F32, tag="ps", name=f"s{qi}")
            passes = [(qh_sb, kh_sb), (qh_sb, kl_sb), (ql_sb, kh_sb)]
        for pi, (qs, ks) in enumerate(passes):
            first = (qi >= RAMP and pi == 0)
            last = pi == len(passes) - 1
            for t in range(NT):
                for kb in range(2):
                    nc.tensor.matmul(
                        ps[:, kb * 512:(kb + 1) * 512],
                        lhsT=qs[:, t, :, qi * 128:(qi + 1) * 128],
                        rhs=ks[:, t, :, kb * 512:(kb + 1) * 512],
                        start=(first and t == 0),
                        stop=(last and t == NT - 1), perf_mode=DR,
                    )
        nc.vector.tensor_reduce(
            out=negm[:, qi:qi + 1], in_=ps[:, :],
            axis=mybir.AxisListType.X, op=mybir.AluOpType.max, negate=True)
        # E8 = fp8(exp(S - m_loc)); Z8 = row-sum POST-quantization
        # (identity tensor_scalar on Pool re-reads the quantized e8)
        nc.scalar.activation(
            out=e8[:, qi, :], in_=ps[:, :],
            func=mybir.ActivationFunctionType.Exp,
            bias=negm[:, qi:qi + 1], scale=1.0)
        if qi % 2 == 0 or not K_Z8:
            nc.scalar.activation(
                out=e8[:, qi, :], in_=e8[:, qi, :],
                func=mybir.ActivationFunctionType.Copy,
                accum_out=zloc[:, qi:qi + 1])
        else:
            nc.vector.tensor_scalar(
                out=e8[:, qi, :], in0=e8[:, qi, :], scalar1=0.0,
                scalar2=1.0, op0=mybir.AluOpType.add,
                op1=mybir.AluOpType.mult,
                accum_out=zloc[:, qi:qi + 1])

        # V tile add while GEMM1 runs (er = enc||res packed by the host)
        er_t = er_tiles[qi]
        nc.vector.tensor_add(out=v16[:, qi, :], in0=er_t[:, 0:D],
                             in1=er_t[:, D:2 * D])

        if qi + 1 == 8:      # exchange-1: tiles 0-7
            gath1 = _exchange_dmas(nc, P, DRAM, negm, zloc, 0, 8, "x8",
                                   use_collective)
            if K_ER:
                _er_batch(8, 12)
        elif qi + 1 == K_XM:  # exchange-1 math once the gather has landed
            _exchange_math(nc, P, sel_sb, negm, zloc, cs, gath1, 0, 8, "x8")
        elif qi + 1 == 12:   # exchange-2: tiles 8-11
            gath2 = _exchange_dmas(nc, P, DRAM, negm, zloc, 8, 12, "x12",
                                   use_collective)
            if K_ER:
                _er_batch(12, 16)
        # V production for exchange-1 tiles, spread so no engine FIFO
        # carries more than ~1 extra bulk op per tile
        VMAP = {13: (0,), 14: (1, 2), 15: (3, 4)}
        for qj in VMAP.get(qi, ()):
            _emit_vprod(nc, v16, vhi, vlo, cs, qj)

    # Post-GEMM1 emission order is latency-tuned: exchange-3's DMA chain
    # first (fires on z8_15, runs while the bursts drain), exchange-2's
    # math ahead of the cs1-gated vlo ops (it is short and unblocks cs2),
    # each burst right after its cs.
    gath3 = _exchange_dmas(nc, P, DRAM, negm, zloc, 12, NQT, "z",
                           use_collective)
    _exchange_math(nc, P, sel_sb, negm, zloc, cs, gath2, 8, 12, "x12")
    _exchange_math(nc, P, sel_sb, negm, zloc, cs, gath3, 12, NQT, "z")
    for qj in (5, 6, 7):
        _emit_vprod(nc, v16, vhi, vlo, cs, qj)
    for qj in range(8, 12):
        _emit_vprod(nc, v16, vhi, vlo, cs, qj)
    for qj in range(12, NQT):
        _emit_vprod(nc, v16, vhi, vlo, cs, qj)

    # ---- GEMM2: out[k, d] = sum_q E8[q, k] * V'c[q, d] ----------
    # per ki one [128,1024] psum tile = two 512-wide groups; groups
    # accumulate q-pair phases following SPLITS so each phase's V tiles
    # are ready (stats exchanged) before its matmuls issue.
    phases = [p // 2 for p in [0] + list(SPLITS) + [NQT]]  # in q-pairs
    ki_sets = [range(0, 4), range(4, 7), range(7, 8)]

    def _copy(idx, out_ap, in_ap):
        # Pool/GPSIMD cannot read PSUM; alternate DVE / ACT
        if idx % 2 == 0:
            nc.vector.tensor_copy(out=out_ap, in_=in_ap)
        else:
            nc.scalar.copy(out=out_ap, in_=in_ap)
    for kis in ki_sets:
        final_set = kis is ki_sets[-1]
        psg = {}
        for pi in range(len(phases) - 1):
            last_phase = pi == len(phases) - 2
            for ki in kis:
                if pi == 0:
                    psg[ki] = PS.tile([128, S2L], F32, tag="ps",
                                      name=f"o{ki}")
                    if final_set:
                        psg["b"] = PS.tile([128, S2L], F32, tag="ps",
                                           name=f"o{ki}b")
                for db in range(2):
                    tgt = psg["b"] if (final_set and db == 1) else psg[ki]
                    for vv in (vhi, vlo):
                        for qp in range(phases[pi], phases[pi + 1]):
                            nc.tensor.matmul(
                                tgt[:, db * 512:(db + 1) * 512],
                                lhsT=e8[:, 2 * qp:2 * qp + 2,
                                        ki * 128:(ki + 1) * 128],
                                rhs=vv[:, 2 * qp:2 * qp + 2,
                                       db * 512:(db + 1) * 512],
                                start=(vv is vhi and qp == 0),
                                stop=(vv is vlo and qp == NQP - 1),
                                perf_mode=DR,
                            )
                    if last_phase:
                        # copy+store while later matmuls still run
                        if db == 0:
                            ot = OST.tile([128, D], FP16, tag="ot",
                                          name=f"ot{ki}")
                        _copy(2 * ki + db, ot[:, db * 512:(db + 1) * 512],
                              tgt[:, db * 512:(db + 1) * 512])
                        eng = (nc.scalar if (final_set and db == 1)
                               else nc.sync)
                        eng.dma_start(
                            out=out[ki * 128:(ki + 1) * 128,
                                    db * 512:(db + 1) * 512],
                            in_=ot[:, db * 512:(db + 1) * 512])


def _build_kernel(nc, qh, ql, kh, kl, er, sel, out, reps=1,
                  use_collective=True):
    tc = tile.TileContext(nc)
    with tc:
        with (
            tc.tile_pool(name="persist", bufs=1) as P,
            tc.tile_pool(name="stage", bufs=8) as ST,
            tc.tile_pool(name="psum", bufs=4, space="PSUM") as PS,
            tc.tile_pool(name="outst", bufs=6) as OST,
            tc.tile_pool(name="dram", bufs=1, space="DRAM") as DRAM,
        ):
            pools = (P, ST, PS, OST, DRAM)
            for _ in range(reps):
                _emit_body(nc, tc, pools, qh, ql, kh, kl, er, sel,
                           out, use_collective)
    return nc


def build(reps=1, use_collective=True):
    nc = bacc.Bacc("TRN2", target_bir_lowering=False, debug=False,
                   num_devices=N_CORES)
    qh = nc.dram_tensor("qh", [128, NT, 2, S], FP8, kind="ExternalInput").ap()
    ql = nc.dram_tensor("ql", [128, NT, 2, S], FP8, kind="ExternalInput").ap()
    kh = nc.dram_tensor("kh", [128, NT, 2, S2L], FP8,
                        kind="ExternalInput").ap()
    kl = nc.dram_tensor("kl", [128, NT, 2, S2L], FP8,
                        kind="ExternalInput").ap()
    er = nc.dram_tensor("er", [S, 2 * D], FP16, kind="ExternalInput").ap()
    sel = nc.dram_tensor("sel", [128, N_CORES], F32,
                         kind="ExternalInput").ap()
    out = nc.dram_tensor("out", [S2L, D], FP16, kind="ExternalOutput").ap()
    _build_kernel(nc, qh, ql, kh, kl, er, sel, out, reps=reps,
                  use_collective=use_collective)
    nc.compile()
    return nc


def _split8(x):
    """fp8 hi + residual lo parts (RNE), as float8_e4m3 arrays."""
    x = np.asarray(x, dtype=np.float32)
    hi = x.astype(E4NP)
    lo = (x - hi.astype(np.float32)).astype(E4NP)
    return hi, lo


def _pack_dr(xT):
    """[D, N] -> [128, D/256, 2, N] DoubleRow pair layout (d = t*256+j*128+p)."""
    Dd, N = xT.shape
    return np.ascontiguousarray(
        xT.reshape(Dd // 256, 2, 128, N).transpose(2, 0, 1, 3))


def make_in_maps(enc_outputs, atten_outputs, enc_residual):
    enc_outputs = np.asarray(enc_outputs, dtype=np.float32)
    atten_outputs = np.asarray(atten_outputs, dtype=np.float32)
    enc_residual = np.asarray(enc_residual, dtype=np.float32)
    er16 = np.concatenate([enc_outputs.astype(np.float16),
                           enc_residual.astype(np.float16)], axis=2)
    in_maps = []
    for core in range(N_CORES):
        b, half = core // 2, core % 2
        sel = np.zeros((128, N_CORES), np.float32)
        sel[:, core ^ 1] = 1.0
        qT = np.ascontiguousarray(enc_outputs[b].T)
        kT = np.ascontiguousarray(
            atten_outputs[b, half * S2L:(half + 1) * S2L, :].T)
        qhi, qlo = _split8(qT)
        khi, klo = _split8(kT)
        in_maps.append({
            "qh": _pack_dr(qhi), "ql": _pack_dr(qlo),
            "kh": _pack_dr(khi), "kl": _pack_dr(klo),
            "er": np.ascontiguousarray(er16[b]),
            "sel": sel,
        })
    return in_maps


def assemble(results):
    out = np.empty((B, S, D), np.float32)
    for core in range(N_CORES):
        b, half = core // 2, core % 2
        out[b, half * S2L:(half + 1) * S2L, :] = (
            results[core]["out"].astype(np.float32))
    return out


_NC = None


def kernel(enc_outputs, atten_outputs, enc_residual):
    global _NC
    if _NC is None:
        _NC = build()
    in_maps = make_in_maps(enc_outputs, atten_outputs, enc_residual)
    last_err = None
    for _attempt in range(3):
        try:
            res = run_bass_kernel_spmd(_NC, in_maps,
                                       core_ids=list(range(N_CORES)))
            return assemble(res.results)
        except Exception as e:  # transient device/tunnel errors -- retry
            last_err = e
    raise last_err
